# revision 21
# baseline (speedup 1.0000x reference)
"""ModalityUntiedAttention on 8 TRN2 NeuronCores (Bass/Tile).

Sharding: 8-way tensor-parallel over heads; every core processes BOTH
batches (2 q heads + 1 kv head per core). This makes both batches'
modality boundaries compile-time constants of one SPMD program.

Expert (modality) routing: tokens are sorted by modality WITHIN each
512-token attention group (host-side permutation), so each group is
[n0 expert-0 tokens | 512-n0 expert-1 tokens] with n0 a compile-time
constant. QKV projections run with the WEIGHTS STATIONARY and the
tokens as the moving operand: each token streams through exactly its
own expert's weights, so mixed-modality groups cost the same as pure
ones (no expert double-compute), and the outputs land directly in
transposed (hd, tok) layout - no Q/K transposes. RMS statistics and
the softmax denominator use gpsimd partition_all_reduce instead of
ones-vector matmuls, keeping the tensor engine for real work.

Attention stays exact: group pairs below the diagonal are fully
causal-allowed; in-group causal masks are host-computed for the
permuted order; the host un-permutes the output rows. Softmax runs
without max subtraction (|scores| <= sqrt(128) for unit-norm rms
weights); attention probs are pre-reduced in 4-key-tile chunks on the
vector engine for the denominator. The wo projection partial sums are
ReduceScattered (bf16) over all 8 cores in 512-token chunks and
RMSNormed on device; per-chunk final norms and per-head denominator
normalization are emitted deferred so no engine queue blocks on a
collective or a long dependency chain.
"""
import sys

sys.path.insert(0, '/opt/trn_rl_repo')

from contextlib import ExitStack

import numpy as np
import ml_dtypes

import concourse.bass as bass
import concourse.tile as tile
from concourse import bacc, mybir, bass_isa
from concourse.bass import ts, ds, _add_dep_helper
from concourse.bass_utils import run_bass_kernel_spmd
from concourse.masks import make_identity

F32 = mybir.dt.float32
BF16 = mybir.dt.bfloat16

E = 2
HQ = 16
HK = 8
HD = 128
DIM = 2048
BS = 2
SEQ = 2048
EPS = 1e-6

N_CORES = 8
HQC = HQ // N_CORES        # 2 q heads per core
HKC = HK // N_CORES        # 1 kv head per core
DQ = HQC * HD              # 256 q cols per core
DKV = HKC * HD             # 128 k (and v) cols per core
NT = SEQ // 128            # 16 token tiles per batch
KT = DIM // 128            # 16 contraction tiles
NG = 4                     # 512-token attention groups per batch
NCHUNK = BS * NG           # 8 (batch, group) chunks
GROUPS = [[0, 1, 2, 3, 4, 5, 6, 7]]

_BUILD_CACHE = {}

MUL = mybir.AluOpType.mult
ADD = mybir.AluOpType.add


def build_nc(has_qkw: bool, has_anw: bool, n0s: tuple):
    """n0s[b*NG+g] = count of modality-0 tokens in group g of batch b."""
    nc = bacc.Bacc("TRN2", target_bir_lowering=False, debug=False,
                   num_devices=N_CORES)

    xTg = nc.dram_tensor("xTg", [NCHUNK, 128, KT, 512], BF16,
                         kind="ExternalInput")
    w0 = nc.dram_tensor("w0", [DIM, DQ + 2 * DKV], BF16, kind="ExternalInput")
    w1 = nc.dram_tensor("w1", [DIM, DQ + 2 * DKV], BF16, kind="ExternalInput")
    wo0 = nc.dram_tensor("wo0", [DQ, DIM], BF16, kind="ExternalInput")
    wo1 = nc.dram_tensor("wo1", [DQ, DIM], BF16, kind="ExternalInput")
    cosT = nc.dram_tensor("cosT", [BS, 128, SEQ], F32, kind="ExternalInput")
    sinT = nc.dram_tensor("sinT", [BS, 128, SEQ], F32, kind="ExternalInput")
    mpc = nc.dram_tensor("mpc", [128, BS * NT], F32, kind="ExternalInput")
    mpc1 = nc.dram_tensor("mpc1", [128, BS * NT], F32, kind="ExternalInput")
    dmin = nc.dram_tensor("dmin", [BS * NT, 128, 512], BF16,
                          kind="ExternalInput")
    if has_qkw:
        qkwT = nc.dram_tensor("qkwT", [BS * 4, 128, SEQ], F32,
                              kind="ExternalInput")
    if has_anw:
        anw0 = nc.dram_tensor("anw0", [1, DIM], F32, kind="ExternalInput")
        anwd = nc.dram_tensor("anwd", [1, DIM], F32, kind="ExternalInput")
        mfin = nc.dram_tensor("mfin", [64, NCHUNK], F32, kind="ExternalInput")

    out_dram = nc.dram_tensor("out", [NCHUNK * 64, DIM], F32,
                              kind="ExternalOutput")

    with tile.TileContext(nc) as tc:
        with ExitStack() as ctx:
            const = ctx.enter_context(tc.tile_pool(name="const", bufs=1))
            persist = ctx.enter_context(tc.tile_pool(name="persist", bufs=1))
            dram = ctx.enter_context(tc.tile_pool(name="dram", bufs=1, space="DRAM"))

            ident = const.tile([128, 128], F32)
            make_identity(nc, ident[:])
            ident_bf = const.tile([128, 128], BF16)
            nc.scalar.copy(ident_bf[:], ident[:])
            mpc_sb = const.tile([128, BS * NT], F32)
            nc.sync.dma_start(mpc_sb[:], mpc[:, :])
            mpc1_sb = const.tile([128, BS * NT], F32)
            nc.sync.dma_start(mpc1_sb[:], mpc1[:, :])
            eps_1 = const.tile([128, 1], F32)
            nc.gpsimd.memset(eps_1[:], float(EPS))
            eps_q = const.tile([128, 1], F32)
            nc.gpsimd.memset(eps_q[:], float(128.0 * EPS))
            dmasks = const.tile([128, BS * NT, 512], BF16)

            # persistent activation buffers (bf16)
            QT = persist.tile([128, HQC, BS, SEQ], BF16)   # q^T (hd, tok)
            KTb = persist.tile([128, BS, SEQ], BF16)       # k^T (hd, tok)
            Vb = persist.tile([128, BS, NT, DKV], BF16)    # v natural (tok, hd)

            # ------------- Phase 1: QKV projection + norms + rope ------------
            with ExitStack() as p1:
                wpool = p1.enter_context(tc.tile_pool(name="wpool", bufs=1))
                ropep = p1.enter_context(tc.tile_pool(name="ropep", bufs=1))
                xpool = p1.enter_context(tc.tile_pool(name="xpool", bufs=2))
                qkps = p1.enter_context(tc.tile_pool(name="qkps", bufs=6, space="PSUM"))
                tps = p1.enter_context(tc.tile_pool(name="tps", bufs=2, space="PSUM"))
                work = p1.enter_context(tc.tile_pool(name="work", bufs=2))

                w0_sb = wpool.tile([128, KT, DQ + 2 * DKV], BF16)
                w1_sb = wpool.tile([128, KT, DQ + 2 * DKV], BF16)
                w0_r = w0.ap().rearrange("(k p) f -> p k f", p=128)
                w1_r = w1.ap().rearrange("(k p) f -> p k f", p=128)
                for k in range(KT):
                    nc.gpsimd.dma_start(w0_sb[:, k, :], w0_r[:, k, :])
                    nc.gpsimd.dma_start(w1_sb[:, k, :], w1_r[:, k, :])
                cos_sb = ropep.tile([128, BS, SEQ], F32)
                nc.gpsimd.dma_start(cos_sb[:], cosT.ap().rearrange("b p t -> p b t"))
                sin_sb = ropep.tile([128, BS, SEQ], F32)
                nc.gpsimd.dma_start(sin_sb[:], sinT.ap().rearrange("b p t -> p b t"))
                if has_qkw:
                    qkw_sb = ropep.tile([128, BS * 4, SEQ], F32)
                    nc.gpsimd.dma_start(
                        qkw_sb[:], qkwT.ap().rearrange("b p t -> p b t"))

                for c in range(NCHUNK):
                    b, g = divmod(c, NG)
                    n0 = n0s[c]
                    xsb = xpool.tile([128, KT, 512], BF16, tag="xt")
                    nc.sync.dma_start(xsb[:], xTg.ap()[c])

                    ps = [qkps.tile([128, 512], F32, tag="qk", name=f"ps{i}")
                          for i in range(4)]
                    for m in range(4):
                        for e, wsb in ((0, w0_sb), (1, w1_sb)):
                            r0, r1 = (0, n0) if e == 0 else (n0, 512)
                            if r1 <= r0:
                                continue
                            for k in range(KT):
                                nc.tensor.matmul(
                                    ps[m][:, r0:r1], wsb[:, k, ts(m, 128)],
                                    xsb[:, k, r0:r1],
                                    start=(k == 0), stop=(k == KT - 1))

                    # q0, q1, k: rms stats + rope, directly in (hd, tok)
                    for hh in range(3):
                        q_sb = work.tile([128, 512], F32, tag="q_sb")
                        nc.scalar.copy(q_sb[:], ps[hh][:])
                        scr = work.tile([128, 512], BF16, tag="scr")
                        nc.scalar.activation(
                            scr[:], ps[hh][:],
                            mybir.ActivationFunctionType.Square)
                        msq = work.tile([128, 512], F32, tag="msq")
                        nc.gpsimd.partition_all_reduce(
                            msq[:], scr[:], channels=128,
                            reduce_op=bass_isa.ReduceOp.add)
                        # q heads fold the 1/sqrt(HD) score scale into the rms
                        # factor: 1/sqrt(sum q^2 + 128 eps) = rms/sqrt(128)
                        sq = work.tile([128, 512], F32, tag="sq")
                        if hh < 2:
                            nc.scalar.activation(
                                sq[:], msq[:],
                                mybir.ActivationFunctionType.Sqrt,
                                scale=1.0, bias=eps_q[:])
                        else:
                            nc.scalar.activation(
                                sq[:], msq[:],
                                mybir.ActivationFunctionType.Sqrt,
                                scale=1.0 / 128.0, bias=eps_1[:])
                        rinv = work.tile([128, 512], F32, tag="rinv")
                        nc.vector.reciprocal_approx_fast(rinv[:], sq[:])

                        # rope in (hd, tok) layout: out[p] = q[p]*cs[p] +
                        # (q*ss2)[p^64]; engines need aligned partitions, so
                        # the half-swap is two small SBUF->SBUF DMAs
                        cs = work.tile([128, 512], F32, tag="cs")
                        nc.vector.tensor_mul(
                            cs[:], cos_sb[:, b, ts(g, 512)], rinv[:])
                        ss = work.tile([128, 512], F32, tag="ss")
                        nc.vector.tensor_mul(
                            ss[:], sin_sb[:, b, ts(g, 512)], rinv[:])
                        if has_qkw:
                            wslice = qkw_sb[:, 2 * b + (hh == 2), ts(g, 512)]
                            wsw = qkw_sb[:, 4 + 2 * b + (hh == 2), ts(g, 512)]
                            nc.vector.tensor_mul(cs[:], cs[:], wslice)
                            nc.vector.tensor_mul(ss[:], ss[:], wsw)
                        ta = work.tile([128, 512], F32, tag="ta")
                        nc.vector.tensor_mul(ta[:], q_sb[:], cs[:])
                        u = work.tile([128, 512], F32, tag="u")
                        nc.vector.tensor_mul(u[:], q_sb[:], ss[:])
                        usw = work.tile([128, 512], F32, tag="usw")
                        nc.sync.dma_start(usw[0:64, :], u[64:128, :])
                        nc.sync.dma_start(usw[64:128, :], u[0:64, :])
                        dst = (QT[:, hh, b, ts(g, 512)] if hh < 2
                               else KTb[:, b, ts(g, 512)])
                        nc.vector.tensor_add(dst, ta[:], usw[:])

                    # v: evict transposed staging, transpose back to natural
                    vt_sb = work.tile([128, 512], BF16, tag="vt_sb")
                    nc.scalar.copy(vt_sb[:], ps[3][:])
                    for u in range(4):
                        tp = tps.tile([128, 128], BF16, tag="tp")
                        nc.tensor.transpose(tp[:], vt_sb[:, ts(u, 128)], ident_bf[:])
                        nc.scalar.copy(Vb[:, b, 4 * g + u, :], tp[:])

            # ------------- Phase 2+3: attention + wo + RS + final norm -------
            with ExitStack() as p23:
                wopool = p23.enter_context(tc.tile_pool(name="wopool", bufs=1))
                ofp = p23.enter_context(tc.tile_pool(name="ofp", bufs=1))
                sps = p23.enter_context(tc.tile_pool(name="sps", bufs=2, space="PSUM"))
                otps = p23.enter_context(tc.tile_pool(name="otps", bufs=2, space="PSUM"))
                wops = p23.enter_context(tc.tile_pool(name="wops", bufs=2, space="PSUM"))
                probs = p23.enter_context(tc.tile_pool(name="probs", bufs=8))
                redc = p23.enter_context(tc.tile_pool(name="redc", bufs=3))
                att = p23.enter_context(tc.tile_pool(name="att", bufs=2))
                opool = p23.enter_context(tc.tile_pool(name="opool", bufs=2))
                npool = p23.enter_context(tc.tile_pool(name="npool", bufs=2))

                ofT = ofp.tile([128, BS, HQC, SEQ], BF16)   # out_flat^T
                nc.gpsimd.dma_start(dmasks[:], dmin.ap().rearrange("t p f -> p t f"))

                wo0_sb = wopool.tile([128, HQC, DIM], BF16)
                nc.sync.dma_start(wo0_sb[:], wo0.ap().rearrange("(k p) f -> p k f", p=128))
                wo1_sb = wopool.tile([128, HQC, DIM], BF16)
                nc.sync.dma_start(wo1_sb[:], wo1.ap().rearrange("(k p) f -> p k f", p=128))
                if has_anw:
                    anw0_sb = wopool.tile([1, DIM], F32)
                    nc.sync.dma_start(anw0_sb[:], anw0[:, :])
                    anwd_sb = wopool.tile([1, DIM], F32)
                    nc.sync.dma_start(anwd_sb[:], anwd[:, :])
                    anw0_b = wopool.tile([128, DIM], F32)
                    nc.gpsimd.partition_broadcast(anw0_b[:], anw0_sb[:])
                    anwd_b = wopool.tile([128, DIM], F32)
                    nc.gpsimd.partition_broadcast(anwd_b[:], anwd_sb[:])
                    mfin_sb = wopool.tile([64, NCHUNK], F32)
                    nc.sync.dma_start(mfin_sb[:], mfin[:, :])

                pending_rs = []
                pending_den = []

                def do_den(b, h, g, acc):
                    dnb = att.tile([128, 512], F32, tag="dnb")
                    nc.gpsimd.partition_all_reduce(
                        dnb[:], acc[:], channels=128,
                        reduce_op=bass_isa.ReduceOp.add)
                    den = att.tile([128, 512], F32, tag="den")
                    nc.vector.reciprocal_approx_fast(den[:], dnb[:])
                    nc.vector.tensor_mul(
                        ofT[:, b, h, ts(g, 512)], ofT[:, b, h, ts(g, 512)],
                        den[:])

                def do_final_norm(cid, rs_out, dep=None):
                    sum_sb = npool.tile([64, DIM], BF16, tag="sum_sb")
                    first = nc.sync.dma_start(sum_sb[:], rs_out[:])
                    if dep is not None:
                        _add_dep_helper(first.ins, dep.ins, sync=False,
                                        reason="defer norm past next chunk")
                    fin = npool.tile([64, DIM], F32, tag="fin")
                    z = npool.tile([64, 1], F32, tag="z")
                    nc.vector.scalar_tensor_tensor(
                        out=fin[:], in0=sum_sb[:], scalar=1.0, in1=sum_sb[:],
                        op0=MUL, op1=MUL, accum_out=z[:])
                    sz = npool.tile([64, 1], F32, tag="sz")
                    nc.scalar.activation(sz[:], z[:],
                                         mybir.ActivationFunctionType.Sqrt,
                                         scale=1.0 / float(DIM), bias=eps_1[0:64, :])
                    rz = npool.tile([64, 1], F32, tag="rz")
                    nc.vector.reciprocal_approx_fast(rz[:], sz[:])
                    nc.scalar.mul(fin[:], sum_sb[:], rz[:])
                    if has_anw:
                        anw_sel = npool.tile([64, DIM], F32, tag="anw_sel")
                        nc.vector.scalar_tensor_tensor(
                            out=anw_sel[:], in0=anwd_b[0:64, :],
                            scalar=mfin_sb[:, cid:cid + 1],
                            in1=anw0_b[0:64, :], op0=MUL, op1=ADD)
                        nc.vector.tensor_mul(fin[:], fin[:], anw_sel[:])
                    nc.sync.dma_start(out_dram.ap()[ts(cid, 64), :], fin[:])

                for c in range(NCHUNK):
                    b, g = divmod(c, NG)
                    n0 = n0s[c]
                    for h in range(HQC):
                        njt = 4 * (g + 1)
                        ot_ps = otps.tile([128, 512], F32, tag="ot")
                        acc = att.tile([128, 512], F32, tag="acc")
                        pp_hold = None
                        for jp in range(njt // 2):
                            j0 = 2 * jp
                            # two 512-score tiles in one 2-bank psum tile so
                            # exp and the causal-mask multiply run 1024 wide
                            s_ps = sps.tile([128, 2, 512], F32, tag="s")
                            for dj in range(2):
                                nc.tensor.matmul(
                                    s_ps[:, dj, :], KTb[:, b, ts(j0 + dj, 128)],
                                    QT[:, h, b, ts(g, 512)],
                                    start=True, stop=True)
                            p_t = probs.tile([128, 2, 512], BF16, tag="p")
                            nc.scalar.activation(
                                p_t[:], s_ps[:], mybir.ActivationFunctionType.Exp)
                            if j0 >= 4 * g:
                                pm_t = probs.tile([128, 2, 512], BF16, tag="pm")
                                nc.vector.tensor_mul(
                                    pm_t[:], p_t[:],
                                    dmasks[:, ds(b * NT + j0, 2), :])
                                p_t = pm_t
                            # denominator: DVE pair tree, f32 accumulate, one
                            # gpsimd partition reduce per (b,h,g)
                            ps_pair = redc.tile([128, 512], BF16, tag="pp")
                            nc.vector.tensor_add(
                                ps_pair[:], p_t[:, 0, :], p_t[:, 1, :])
                            if jp % 2 == 0:
                                pp_hold = ps_pair
                            elif jp == 1:
                                nc.vector.tensor_add(
                                    acc[:], pp_hold[:], ps_pair[:])
                            else:
                                ps4 = redc.tile([128, 512], BF16, tag="p4")
                                nc.vector.tensor_add(
                                    ps4[:], pp_hold[:], ps_pair[:])
                                nc.vector.tensor_add(acc[:], acc[:], ps4[:])
                            for dj in range(2):
                                j = j0 + dj
                                nc.tensor.matmul(
                                    ot_ps[:], Vb[:, b, j, :], p_t[:, dj, :],
                                    start=(j == 0), stop=(j == njt - 1))
                        # fast raw evict frees the psum; normalization deferred
                        nc.vector.tensor_copy(ofT[:, b, h, ts(g, 512)], ot_ps[:])
                        pending_den.append((b, h, g, acc))
                        if len(pending_den) > 1:
                            do_den(*pending_den.pop(0))

                    while pending_den:
                        do_den(*pending_den.pop(0))

                    # wo projection for this 512-token chunk
                    rs_in = dram.tile([512, DIM], BF16, tag="rs_in", bufs=2)
                    for u in range(4):
                        T = 4 * g + u
                        lo, hi = 128 * u, 128 * (u + 1)
                        kind = 0 if n0 >= hi else (1 if n0 <= lo else 2)
                        woa_sb = wo1_sb if kind == 1 else wo0_sb
                        o_sb = opool.tile([128, DIM], BF16, tag="o_sb")
                        for n in range(4):
                            wo_ps = wops.tile([128, 512], F32, tag="wop")
                            if kind == 2:
                                wb_ps = wops.tile([128, 512], F32, tag="wop")
                            for kk in range(HQC):
                                nc.tensor.matmul(
                                    wo_ps[:], ofT[:, b, kk, ts(T, 128)],
                                    woa_sb[:, kk, ts(n, 512)],
                                    start=(kk == 0), stop=(kk == HQC - 1))
                                if kind == 2:
                                    nc.tensor.matmul(
                                        wb_ps[:], ofT[:, b, kk, ts(T, 128)],
                                        wo1_sb[:, kk, ts(n, 512)],
                                        start=(kk == 0), stop=(kk == HQC - 1))
                            if kind == 2:
                                nc.scalar.mul(o_sb[:, ts(n, 512)], wo_ps[:],
                                              mpc1_sb[:, b * NT + T:b * NT + T + 1])
                                nc.vector.scalar_tensor_tensor(
                                    out=o_sb[:, ts(n, 512)], in0=wb_ps[:],
                                    scalar=mpc_sb[:, b * NT + T:b * NT + T + 1],
                                    in1=o_sb[:, ts(n, 512)], op0=MUL, op1=ADD)
                            else:
                                nc.scalar.copy(o_sb[:, ts(n, 512)], wo_ps[:])
                        last_rsin_dma = nc.sync.dma_start(rs_in[ts(u, 128), :], o_sb[:])

                    rs_out = dram.tile([64, DIM], BF16, tag="rs_out", bufs=2)
                    nc.gpsimd.collective_compute(
                        "ReduceScatter", mybir.AluOpType.add,
                        replica_groups=GROUPS,
                        ins=[rs_in.opt()], outs=[rs_out.opt()])
                    pending_rs.append((c, rs_out))
                    # final norm for an older chunk: its RS finished while this
                    # chunk computed, so the queues never block on it
                    if len(pending_rs) > 1:
                        pc, prs = pending_rs.pop(0)
                        do_final_norm(pc, prs, dep=last_rsin_dma)

                for pc, prs in pending_rs:
                    do_final_norm(pc, prs)

    nc.compile()
    return nc


def _plan(modality_ids):
    """Per-group stable modality sort; per-(batch,group) boundary counts."""
    mids = np.asarray(modality_ids).reshape(BS, SEQ)
    perms = np.empty((BS, SEQ), np.int64)   # permuted pos -> original token idx
    n0s = []
    for b in range(BS):
        for G in range(NG):
            mg = mids[b, 512 * G:512 * (G + 1)]
            i0 = np.where(mg == 0)[0]
            i1 = np.where(mg == 1)[0]
            n0s.append(len(i0))
            perms[b, 512 * G:512 * (G + 1)] = 512 * G + np.concatenate([i0, i1])
    return perms, tuple(n0s)


def _prep_inputs(x, freqs_cos, freqs_sin, wq, wk, wv, wo,
                 q_norm_w, k_norm_w, attn_norm_w, modality_ids,
                 has_qkw, has_anw, perms, n0s):
    """Build the 8 per-core input maps (numpy marshaling only)."""
    x = np.asarray(x, np.float32)
    freqs_cos = np.asarray(freqs_cos, np.float32)
    freqs_sin = np.asarray(freqs_sin, np.float32)
    wq = np.asarray(wq, np.float32)
    wk = np.asarray(wk, np.float32)
    wv = np.asarray(wv, np.float32)
    wo = np.asarray(wo, np.float32)
    mids = np.asarray(modality_ids).reshape(BS, SEQ)

    # de-interleave the hd dimension: [even dims, odd dims]
    perm_hd = np.concatenate([np.arange(0, HD, 2), np.arange(1, HD, 2)])

    def permute_heads(w, nh):
        w4 = w.reshape(E, DIM, nh, HD)
        return w4[:, :, :, perm_hd].reshape(E, DIM, nh * HD)

    wq_p = permute_heads(wq, HQ)
    wk_p = permute_heads(wk, HK)
    wv_p = permute_heads(wv, HK)
    wo4 = wo.reshape(E, HQ, HD, DIM)[:, :, perm_hd, :].reshape(E, HQ * HD, DIM)

    cosf = np.concatenate([freqs_cos, freqs_cos], axis=1)          # (SEQ, HD)
    # pre-swap sin arrangement: u = q*ss2 is computed per partition, then the
    # hd halves of u are swapped, so rows [0:64]=+sin land at out[64:128]
    sinf = np.concatenate([freqs_sin, -freqs_sin], axis=1)         # (SEQ, HD)

    # shared per-batch marshaling
    xTg_b, cosT_b, sinT_b, dmin_b, m_b = [], [], [], [], []
    for b in range(BS):
        P = perms[b]
        xp = x[b].T[:, P].reshape(KT, 128, NG, 512).transpose(2, 1, 0, 3)
        xTg_b.append(np.ascontiguousarray(xp).astype(ml_dtypes.bfloat16))
        cosT_b.append(np.ascontiguousarray(cosf[P].T))
        sinT_b.append(np.ascontiguousarray(sinf[P].T))
        m_b.append(mids[b].astype(np.float32)[P])
        pos = (P % 512)
        dmv = np.zeros((NT, 128, 512), np.float32)
        for j in range(NT):
            gj = j // 4
            kpos = pos[128 * j:128 * (j + 1)]
            qpos = pos[512 * gj:512 * (gj + 1)]
            dmv[j] = (kpos[:, None] <= qpos[None, :])
        dmin_b.append(dmv.astype(ml_dtypes.bfloat16))

    xTg = np.concatenate(xTg_b).reshape(NCHUNK, 128, KT, 512)
    cosT = np.stack(cosT_b)
    sinT = np.stack(sinT_b)
    dmin = np.concatenate(dmin_b)
    mpc = np.stack([m.reshape(NT, 128).T for m in m_b], axis=1)
    mpc = np.ascontiguousarray(mpc.reshape(128, BS * NT))
    mpc1 = np.ascontiguousarray(1.0 - mpc)

    in_maps = []
    for c in range(N_CORES):
        qs = slice(c * DQ, (c + 1) * DQ)
        ks = slice(c * DKV, (c + 1) * DKV)
        w0c = np.concatenate([wq_p[0][:, qs], wk_p[0][:, ks], wv_p[0][:, ks]], axis=1)
        w1c = np.concatenate([wq_p[1][:, qs], wk_p[1][:, ks], wv_p[1][:, ks]], axis=1)
        im = {
            "xTg": xTg,
            "w0": w0c.astype(ml_dtypes.bfloat16),
            "w1": w1c.astype(ml_dtypes.bfloat16),
            "wo0": wo4[0][qs, :].astype(ml_dtypes.bfloat16),
            "wo1": wo4[1][qs, :].astype(ml_dtypes.bfloat16),
            "cosT": cosT,
            "sinT": sinT,
            "mpc": mpc,
            "mpc1": mpc1,
            "dmin": dmin,
        }
        if has_qkw:
            qw = np.asarray(q_norm_w, np.float32)[:, perm_hd]
            kw = np.asarray(k_norm_w, np.float32)[:, perm_hd]
            hd_sw = np.concatenate([np.arange(64, 128), np.arange(0, 64)])
            qkwT = np.empty((BS * 4, 128, SEQ), np.float32)
            for b in range(BS):
                msel = mids[b][perms[b]]
                qkwT[2 * b + 0] = qw[msel].T
                qkwT[2 * b + 1] = kw[msel].T
                qkwT[4 + 2 * b + 0] = qw[msel].T[hd_sw]
                qkwT[4 + 2 * b + 1] = kw[msel].T[hd_sw]
            im["qkwT"] = qkwT
        if has_anw:
            aw = np.asarray(attn_norm_w, np.float32)
            im["anw0"] = np.ascontiguousarray(aw[0:1])
            im["anwd"] = (aw[1] - aw[0]).reshape(1, DIM).copy()
            mf = np.empty((64, NCHUNK), np.float32)
            for cid in range(NCHUNK):
                b, g = divmod(cid, NG)
                t0 = 512 * g + 64 * c
                mf[:, cid] = m_b[b][t0:t0 + 64]
            im["mfin"] = mf
        in_maps.append(im)
    return in_maps


def kernel(**inputs):
    q_norm_w = np.asarray(inputs["q_norm_w"], np.float32)
    k_norm_w = np.asarray(inputs["k_norm_w"], np.float32)
    attn_norm_w = np.asarray(inputs["attn_norm_w"], np.float32)
    has_qkw = not (np.all(q_norm_w == 1.0) and np.all(k_norm_w == 1.0))
    has_anw = not np.all(attn_norm_w == 1.0)

    perms, n0s = _plan(inputs["modality_ids"])
    key = (has_qkw, has_anw, n0s)
    if key not in _BUILD_CACHE:
        _BUILD_CACHE[key] = build_nc(has_qkw, has_anw, n0s)
    nc = _BUILD_CACHE[key]

    in_maps = _prep_inputs(
        inputs["x"], inputs["freqs_cos"], inputs["freqs_sin"],
        inputs["wq"], inputs["wk"], inputs["wv"], inputs["wo"],
        q_norm_w, k_norm_w, attn_norm_w, inputs["modality_ids"],
        has_qkw, has_anw, perms, n0s)

    res = run_bass_kernel_spmd(nc, in_maps, core_ids=list(range(N_CORES)))

    out = np.empty((BS, SEQ, DIM), np.float32)
    for c in range(N_CORES):
        oc = res.results[c]["out"]          # (NCHUNK*64, DIM), permuted rows
        for cid in range(NCHUNK):
            b, g = divmod(cid, NG)
            t0 = 512 * g + 64 * c           # permuted-space positions
            out[b, perms[b][t0:t0 + 64], :] = oc[64 * cid:64 * (cid + 1), :]
    return out
